# revision 25
# baseline (speedup 1.0000x reference)
"""Trainium2 Bass kernel for nn_BSplineActivation.

Math: y[b,f] = sum_n B_n(x[b,f]) * coeff[f,n], cubic B-splines on the uniform
grid linspace(-1,1,14); x clamped to [-1,1]. Per feature, y is a 13-piece C2
cubic in u = 6.5*x + 6.5 with knots at the integers.

Approximation (rel-L2 ~4e-3, gate is 2e-2): per feature f,
  y ~= a0[f] + ax[f]*xc + axx[f]*xc^2
       + sum_{t=0}^{6} be[t,f]*E_t + sum_{t=0}^{6} bo[t,f]*(sgn01 . E_t)
where xc = clip(x,-1,1), m = min(|x|,1), E_t = erf((6.5*m - t)/0.8) and
sgn01 = 1[xc >= 0]. The 13 half-integer-centered erf ladder rungs that fit a
random spline to ~3e-3 fold in symmetric pairs about u=6.5: 7 even planes
E_t(m) (1 ACT op each) span the even part, and the odd part reuses the SAME
planes through a second PSUM chain multiplied by the sign plane in the tail.
Coefficients are per-feature weighted least squares (Gaussian x-density plus
the clamp point masses at x=+-1, exact-interpolation constraints at the two
endpoints), solved on host per call.

Numerics: every basis plane is a smooth function of x evaluated from fp16
tiles, and all fitted coefficients are O(0.3), so fp16 planes/coeffs perturb
y by ~5e-4 (no cancellation anywhere; the ill-conditioned truncated-power
form never materializes on device). PE matmuls run fp16 (1 cyc/row).

Device layout: features on partitions (8 groups of 128 per core), batch on
the free dim; pure data parallel over batch across 8 cores (hosts pass
feature-major transposed shards). Per group-tile [128,1024]:
  DVE: xc16/m16/sgn01 tensor_scalar planes, xg^2, two stt tails
  ACT: 7 erf planes (fused affine, fp16 out)
  Pool: 16 fp16 diag builds (affine_select)
  PE: 16 diag-matmul chains into two PSUM banks-pairs (C and G)
  out: Y fp16, host upcasts.
"""

import math
import os

import numpy as np

import concourse.bacc as bacc
import concourse.bass as bass
import concourse.mybir as mybir
import concourse.tile as tile
from concourse.bass_utils import run_bass_kernel_spmd

N_CORES = 8
B_FULL, F = 8192, 1024
B_CORE = B_FULL // N_CORES  # 1024
P = 128
G = F // P  # 8 feature groups per core
W = B_CORE  # tile width (batch columns)
HALF = 512  # matmul moving-dim limit

NB13 = 13
NT = 7          # erf ladder rungs after symmetry folding (t = 0..6)
ERF_S = 0.8     # erf smoothing width in u units
ERF_SCALE = float(np.float32(6.5 / ERF_S))   # ACT scale on the m plane
NBASIS = 2 + NT + NT  # xc, xc^2, E_t (C chain), E_t (G chain); const via tail

FP32 = mybir.dt.float32
FP16 = mybir.dt.float16
Alu = mybir.AluOpType
Act = mybir.ActivationFunctionType

_CACHE: dict = {}


def _build_nc() -> bass.Bass:
    nc = bacc.Bacc("TRN2", target_bir_lowering=False, debug=False)

    xT = nc.dram_tensor("xT", [F, B_CORE], FP32, kind="ExternalInput")
    # fp16 coefficient table, per feature-group packed columns:
    #   [g*NBASIS + 0]        ax      (xc chain, C psum)
    #   [g*NBASIS + 1]        axx     (xc^2 chain, C psum)
    #   [g*NBASIS + 2 + t]    be[t]   (E_t chain, C psum)
    #   [g*NBASIS + 2+NT + t] bo[t]   (E_t chain, G psum)
    tabs16 = nc.dram_tensor("tabs16", [P, G * NBASIS], FP16, kind="ExternalInput")
    # fp32 table: erf bias columns [0..NT), a0 per group [NT + g], ones [NT+G]
    tabs32 = nc.dram_tensor("tabs32", [P, NT + G + 1], FP32, kind="ExternalInput")
    yT = nc.dram_tensor("yT", [F, B_CORE], FP16, kind="ExternalOutput")

    with tile.TileContext(nc) as tc:
        with (
            tc.tile_pool(name="const", bufs=1) as const_pool,
            tc.tile_pool(name="xdata", bufs=2) as x_pool,
            tc.tile_pool(name="plane", bufs=1) as pl_pool,
            tc.tile_pool(name="diag", bufs=2) as diag_pool,
            tc.tile_pool(name="yout", bufs=2) as y_pool,
            tc.tile_pool(name="psum", bufs=1, space="PSUM") as psum_pool,
        ):
            # narrow tiles first (fast fill), wide pairs after (amortized ACT
            # per-op overhead)
            SEGS = [[0], [1], [2, 3], [4, 5], [6, 7]]

            Xs = []
            for i, seg in enumerate(SEGS):
                ng = len(seg)
                X = x_pool.tile([P, ng * W], FP32, name=f"X{i}", tag=f"X{i % 2}")
                if ng == 1:
                    nc.sync.dma_start(X[:], xT[seg[0] * P : (seg[0] + 1) * P, :])
                else:
                    nc.sync.dma_start(
                        X[:].rearrange("p (gl b) -> p gl b", gl=ng),
                        xT[seg[0] * P : (seg[0] + ng) * P, :].rearrange(
                            "(gl p) b -> p gl b", p=P),
                    )
                Xs.append(X)
                if i == 0:
                    T16 = const_pool.tile([P, G * NBASIS], FP16, name="T16")
                    T32 = const_pool.tile([P, NT + G + 1], FP32, name="T32")
                    nc.sync.dma_start(T16[:], tabs16[:])
                    nc.sync.dma_start(T32[:], tabs32[:])

            def ccol16(g, k):
                c = g * NBASIS + k
                return T16[:, c : c + 1]

            def emit_tail(g, goff, xc, Cp):
                # tail: Y = (ax*xc + C) + a0 (stt + ts). Deferred one group so
                # the PSUM-waiting ops never stall the DVE plane stream.
                P1 = y_pool.tile([P, W], FP32, name="P1", tag="P1")
                nc.vector.scalar_tensor_tensor(
                    P1[:], xc[:, goff : goff + W],
                    T16[:, g * NBASIS : g * NBASIS + 1],
                    Cp[:], Alu.mult, Alu.add,
                )
                Y = y_pool.tile([P, W], FP16, name="Y", tag="Y")
                nc.vector.tensor_scalar(
                    Y[:], P1[:], T32[:, NT + g : NT + g + 1], None, Alu.add
                )
                nc.sync.dma_start(yT[g * P : (g + 1) * P, :], Y[:])

            def diag16(col, name, tag):
                d = diag_pool.tile([P, P], FP16, name=name, tag=tag)
                nc.gpsimd.affine_select(
                    d[:], col.broadcast_to([P, P]),
                    pattern=[[-1, P]], compare_op=Alu.is_equal,
                    fill=0.0, base=0, channel_multiplier=1,
                )
                return d

            pending_tail = None
            for i, seg in enumerate(SEGS):
                X = Xs[i]
                ng = len(seg)
                SW = ng * W

                # fp16 planes (DVE)
                xc = pl_pool.tile([P, SW], FP16, name="xc", tag="xc")
                nc.vector.tensor_scalar(xc[:], X[:], -1.0, 1.0, Alu.max,
                                        Alu.min)
                cn = pl_pool.tile([P, SW], FP16, name="cn", tag="cn")
                nc.vector.tensor_scalar(cn[:], X[:], -1.0, 1.0, Alu.mult,
                                        Alu.min)
                m = pl_pool.tile([P, SW], FP16, name="m", tag="m")
                nc.vector.tensor_tensor(m[:], xc[:], cn[:], Alu.max)
                x2 = pl_pool.tile([P, SW], FP16, name="x2", tag="x2")
                nc.vector.tensor_tensor(x2[:], xc[:], xc[:], Alu.mult)
                sg = pl_pool.tile([P, SW], FP16, name="sg", tag="sg")
                nc.vector.tensor_scalar(sg[:], xc[:], 0.0, None, Alu.is_ge)

                # erf ladder planes (ACT) and their odd partners (DVE; exact:
                # sg is 0/1 so O = sg*E has no extra rounding)
                E, O = [], []
                for t in range(NT):
                    e = pl_pool.tile([P, SW], FP16, name=f"E{t}", tag=f"E{t}")
                    nc.scalar.activation(
                        e[:], m[:], Act.Erf,
                        scale=ERF_SCALE, bias=T32[:, t : t + 1],
                    )
                    E.append(e)
                    o = pl_pool.tile([P, SW], FP16, name=f"O{t}", tag=f"O{t}")
                    nc.vector.tensor_tensor(o[:], sg[:], e[:], Alu.mult)
                    O.append(o)

                halves = []
                for half, g in enumerate(seg):
                    dxx = diag16(ccol16(g, 1), f"dxx{g}", f"dxx{half}")
                    dE = [diag16(ccol16(g, 2 + t), f"dE{g}_{t}",
                                 f"dE{half}_{t}") for t in range(NT)]
                    dO = [diag16(ccol16(g, 2 + NT + t), f"dO{g}_{t}",
                                 f"dO{half}_{t}") for t in range(NT)]
                    Cp = psum_pool.tile([P, W], FP32, name="Cp",
                                        tag=f"Cp{g % 4}")
                    halves.append((g, half * W, dxx, dE, dO, Cp))

                # single PE chain per group:
                # C = axx*x2 + sum be*E + sum bo*O, interleaved across groups
                # and halves per plane
                for (g, goff, dxx, dE, dO, Cp) in halves:
                    for lo in (0, HALF):
                        nc.tensor.matmul(Cp[:, lo : lo + HALF], dxx[:],
                                         x2[:, goff + lo : goff + lo + HALF],
                                         start=True, stop=False)
                for t in range(NT):
                    for (g, goff, dxx, dE, dO, Cp) in halves:
                        for lo in (0, HALF):
                            sl = slice(lo, lo + HALF)
                            xsl = slice(goff + lo, goff + lo + HALF)
                            nc.tensor.matmul(Cp[:, sl], dE[t][:], E[t][:, xsl],
                                             start=False, stop=False)
                            nc.tensor.matmul(Cp[:, sl], dO[t][:], O[t][:, xsl],
                                             start=False, stop=(t == NT - 1))

                for (g, goff, dxx, dE, dO, Cp) in halves:
                    if pending_tail is not None:
                        emit_tail(*pending_tail)
                    pending_tail = (g, goff, xc, Cp)
            emit_tail(*pending_tail)
    nc.compile()
    return nc


def _exact_spline_d(coeff: np.ndarray) -> np.ndarray:
    """Truncated-power coefficients d[j,f]: y(u) = sum_j d_j relu(u-j)^3."""
    d = np.zeros((NB13, F), dtype=np.float64)
    c64 = coeff.astype(np.float64)
    for j in range(NB13):
        for r in range(5):
            n = j - r
            if 0 <= n < coeff.shape[1]:
                d[j] += (-1) ** r * math.comb(4, r) / 6.0 * c64[:, n]
    return d


def _host_fit(coeff: np.ndarray):
    """Weighted, endpoint-constrained LS fit of the folded-erf basis.

    Returns (tabs16 [P, G*NBASIS] fp16, tabs32 [P, NT+G] fp32).
    """
    d = _exact_spline_d(coeff)
    M = 2601
    ug = np.linspace(0.0, 13.0, M)
    # weight: Gaussian density of u = 6.5 x + 6.5 plus clamp point masses
    z = (ug - 6.5) / 6.5
    w = np.exp(-0.5 * z * z)
    tail = math.erfc(1.0 / math.sqrt(2.0)) / 2.0  # P(x > 1)
    w /= w.sum() / (1.0 - 2.0 * tail)
    w[0] += tail
    w[-1] += tail

    yex = np.zeros((M, F))
    for j in range(NB13):
        yex += np.maximum(ug - j, 0.0)[:, None] ** 3 * d[j][None, :]

    # basis columns, mirroring the device fp16 pipeline
    xg32 = z.astype(np.float32)
    xc = np.clip(xg32, -1.0, 1.0).astype(np.float16)
    mm = np.minimum(np.abs(xg32), 1.0).astype(np.float16)
    sg01 = (xc >= 0).astype(np.float64)
    xcf = xc.astype(np.float32)
    from math import erf as _erf
    cols = [np.ones(M), xcf.astype(np.float64),
            (xcf * xcf).astype(np.float16).astype(np.float64)]
    Ecols = []
    for t in range(NT):
        bias = np.float32(-t / ERF_S)
        arg = np.float32(ERF_SCALE) * mm.astype(np.float32) + bias
        e = np.array([_erf(float(v)) for v in arg], dtype=np.float32)
        e16 = e.astype(np.float16).astype(np.float64)
        Ecols.append(e16)
    cols += Ecols
    cols += [sg01 * e for e in Ecols]
    A = np.stack(cols, axis=1)          # (M, 3 + 2*NT)
    B = A.shape[1]

    sw = np.sqrt(w)
    Aw = A * sw[:, None]
    C2 = A[[0, -1], :]
    yc = yex[[0, -1], :]
    AtA = Aw.T @ Aw
    Atb = Aw.T @ (yex * sw[:, None])
    K = np.block([[AtA, C2.T], [C2, np.zeros((2, 2))]])
    sol = np.linalg.lstsq(K, np.vstack([Atb, yc]), rcond=None)[0][:B]
    # sol rows: [a0, ax, axx, be_0..be_6, bo_0..bo_6], per feature

    t16 = np.zeros((P, G * NBASIS), dtype=np.float16)
    t32 = np.zeros((P, NT + G + 1), dtype=np.float32)
    for t in range(NT):
        t32[:, t] = np.float32(-t / ERF_S)
    t32[:, NT + G] = 1.0
    for g in range(G):
        fsl = slice(g * P, (g + 1) * P)
        t16[:, g * NBASIS + 0] = sol[1, fsl].astype(np.float16)
        t16[:, g * NBASIS + 1] = sol[2, fsl].astype(np.float16)
        for t in range(NT):
            t16[:, g * NBASIS + 2 + t] = sol[3 + t, fsl].astype(np.float16)
            t16[:, g * NBASIS + 2 + NT + t] = sol[3 + NT + t, fsl].astype(
                np.float16)
        t32[:, NT + g] = sol[0, fsl].astype(np.float32)
    return t16, t32


def kernel(x: np.ndarray, coeff: np.ndarray) -> np.ndarray:
    x = np.ascontiguousarray(x, dtype=np.float32)
    coeff = np.ascontiguousarray(coeff, dtype=np.float32)
    assert x.shape == (B_FULL, F) and coeff.shape == (F, 10)

    if "nc" not in _CACHE:
        _CACHE["nc"] = _build_nc()
    nc = _CACHE["nc"]

    tabs16, tabs32 = _host_fit(coeff)

    in_maps = []
    for c in range(N_CORES):
        shard = np.ascontiguousarray(x[c * B_CORE : (c + 1) * B_CORE, :].T)
        in_maps.append({"xT": shard, "tabs16": tabs16, "tabs32": tabs32})

    trace = os.environ.get("BSPLINE_TRACE", "0") == "1"
    res = run_bass_kernel_spmd(
        nc, in_maps, core_ids=list(range(N_CORES)), trace=trace
    )
    _CACHE["last_result"] = res

    y = np.empty((B_FULL, F), dtype=np.float32)
    for c in range(N_CORES):
        y[c * B_CORE : (c + 1) * B_CORE, :] = (
            res.results[c]["yT"].astype(np.float32).T
        )
    return y


# revision 26
# speedup vs baseline: 1.3103x; 1.3103x over previous
"""Trainium2 Bass kernel for nn_BSplineActivation.

Math: y[b,f] = sum_n B_n(x[b,f]) * coeff[f,n], cubic B-splines on the uniform
grid linspace(-1,1,14); x clamped to [-1,1]. Per feature, y is a 13-piece C2
cubic in u = 6.5*x + 6.5 with knots at the integers.

Approximation (rel-L2 ~4e-3, gate is 2e-2): per feature f,
  y ~= a0[f] + ax[f]*xc + axx[f]*xc^2
       + sum_{t=0}^{6} be[t,f]*E_t + sum_{t=0}^{6} bo[t,f]*(sgn01 . E_t)
where xc = clip(x,-1,1), m = min(|x|,1), E_t = erf((6.5*m - t)/0.8) and
sgn01 = 1[xc >= 0]. The 13 half-integer-centered erf ladder rungs that fit a
random spline to ~3e-3 fold in symmetric pairs about u=6.5: 7 even planes
E_t(m) (1 ACT op each) span the even part, and the odd part reuses the SAME
planes through a second PSUM chain multiplied by the sign plane in the tail.
Coefficients are per-feature weighted least squares (Gaussian x-density plus
the clamp point masses at x=+-1, exact-interpolation constraints at the two
endpoints), solved on host per call.

Numerics: every basis plane is a smooth function of x evaluated from fp16
tiles, and all fitted coefficients are O(0.3), so fp16 planes/coeffs perturb
y by ~5e-4 (no cancellation anywhere; the ill-conditioned truncated-power
form never materializes on device). PE matmuls run fp16 (1 cyc/row).

Device layout: features on partitions (8 groups of 128 per core), batch on
the free dim; pure data parallel over batch across 8 cores (hosts pass
feature-major transposed shards). Per group-tile [128,1024]:
  DVE: xc16/m16/sgn01 tensor_scalar planes, xg^2, two stt tails
  ACT: 7 erf planes (fused affine, fp16 out)
  Pool: 16 fp16 diag builds (affine_select)
  PE: 16 diag-matmul chains into two PSUM banks-pairs (C and G)
  out: Y fp16, host upcasts.
"""

import math
import os

import numpy as np

import concourse.bacc as bacc
import concourse.bass as bass
import concourse.mybir as mybir
import concourse.tile as tile
from concourse.bass_utils import run_bass_kernel_spmd

N_CORES = 8
B_FULL, F = 8192, 1024
B_CORE = B_FULL // N_CORES  # 1024
P = 128
G = F // P  # 8 feature groups per core
W = B_CORE  # tile width (batch columns)
HALF = 512  # matmul moving-dim limit

NB13 = 13
NT = 7          # erf ladder rungs after symmetry folding (t = 0..6)
ERF_S = 0.8     # erf smoothing width in u units
ERF_SCALE = float(np.float32(6.5 / ERF_S))   # ACT scale on the m plane
NBASIS = 2 + NT + NT  # xc, xc^2, E_t (C chain), E_t (G chain); const via tail

FP32 = mybir.dt.float32
FP16 = mybir.dt.float16
Alu = mybir.AluOpType
Act = mybir.ActivationFunctionType

_CACHE: dict = {}


def _build_nc() -> bass.Bass:
    nc = bacc.Bacc("TRN2", target_bir_lowering=False, debug=False)

    xT = nc.dram_tensor("xT", [F, B_CORE], FP32, kind="ExternalInput")
    # fp16 coefficient table, per feature-group packed columns:
    #   [g*NBASIS + 0]        ax      (xc chain, C psum)
    #   [g*NBASIS + 1]        axx     (xc^2 chain, C psum)
    #   [g*NBASIS + 2 + t]    be[t]   (E_t chain, C psum)
    #   [g*NBASIS + 2+NT + t] bo[t]   (E_t chain, G psum)
    tabs16 = nc.dram_tensor("tabs16", [P, G * NBASIS], FP16, kind="ExternalInput")
    # fp32 table: erf bias columns [0..NT), a0 per group [NT + g], ones [NT+G]
    tabs32 = nc.dram_tensor("tabs32", [P, NT + G + 1], FP32, kind="ExternalInput")
    yT = nc.dram_tensor("yT", [F, B_CORE], FP16, kind="ExternalOutput")

    with tile.TileContext(nc) as tc:
        with (
            tc.tile_pool(name="const", bufs=1) as const_pool,
            tc.tile_pool(name="xdata", bufs=1) as x_pool,
            tc.tile_pool(name="plane", bufs=2) as pl_pool,
            tc.tile_pool(name="diag", bufs=2) as diag_pool,
            tc.tile_pool(name="yout", bufs=2) as y_pool,
            tc.tile_pool(name="psum", bufs=1, space="PSUM") as psum_pool,
        ):
            # narrow tiles first (fast fill), wide pairs after (amortized ACT
            # per-op overhead)
            SEGS = [[0], [1], [2, 3], [4, 5], [6, 7]]

            Xs = []
            for i, seg in enumerate(SEGS):
                ng = len(seg)
                X = x_pool.tile([P, ng * W], FP32, name=f"X{i}", tag=f"X{i % 2}")
                if ng == 1:
                    nc.sync.dma_start(X[:], xT[seg[0] * P : (seg[0] + 1) * P, :])
                else:
                    nc.sync.dma_start(
                        X[:].rearrange("p (gl b) -> p gl b", gl=ng),
                        xT[seg[0] * P : (seg[0] + ng) * P, :].rearrange(
                            "(gl p) b -> p gl b", p=P),
                    )
                Xs.append(X)
                if i == 0:
                    T16 = const_pool.tile([P, G * NBASIS], FP16, name="T16")
                    T32 = const_pool.tile([P, NT + G + 1], FP32, name="T32")
                    nc.sync.dma_start(T16[:], tabs16[:])
                    nc.sync.dma_start(T32[:], tabs32[:])

            def ccol16(g, k):
                c = g * NBASIS + k
                return T16[:, c : c + 1]

            def emit_tail(g, goff, xc, Cp):
                # tail: Y = (ax*xc + C) + a0 (stt + ts). Deferred one group so
                # the PSUM-waiting ops never stall the DVE plane stream.
                P1 = y_pool.tile([P, W], FP32, name="P1", tag="P1")
                nc.vector.scalar_tensor_tensor(
                    P1[:], xc[:, goff : goff + W],
                    T16[:, g * NBASIS : g * NBASIS + 1],
                    Cp[:], Alu.mult, Alu.add,
                )
                Y = y_pool.tile([P, W], FP16, name="Y", tag="Y")
                nc.vector.tensor_scalar(
                    Y[:], P1[:], T32[:, NT + g : NT + g + 1], None, Alu.add
                )
                nc.sync.dma_start(yT[g * P : (g + 1) * P, :], Y[:])

            def diag16(col, name, tag):
                d = diag_pool.tile([P, P], FP16, name=name, tag=tag)
                nc.gpsimd.affine_select(
                    d[:], col.broadcast_to([P, P]),
                    pattern=[[-1, P]], compare_op=Alu.is_equal,
                    fill=0.0, base=0, channel_multiplier=1,
                )
                return d

            pending_tail = None
            for i, seg in enumerate(SEGS):
                X = Xs[i]
                ng = len(seg)
                SW = ng * W

                # fp16 planes (DVE)
                xc = pl_pool.tile([P, SW], FP16, name="xc", tag="xc")
                nc.vector.tensor_scalar(xc[:], X[:], -1.0, 1.0, Alu.max,
                                        Alu.min)
                m = pl_pool.tile([P, SW], FP16, name="m", tag="m")
                nc.vector.tensor_scalar(m[:], X[:], -1.0, 1.0, Alu.mult,
                                        Alu.min)
                nc.vector.tensor_tensor(m[:], xc[:], m[:], Alu.max)
                x2 = pl_pool.tile([P, SW], FP16, name="x2", tag="x2")
                nc.vector.tensor_tensor(x2[:], xc[:], xc[:], Alu.mult)
                sg = pl_pool.tile([P, SW], FP16, name="sg", tag="sg")
                nc.vector.tensor_scalar(sg[:], xc[:], 0.0, None, Alu.is_ge)

                # erf ladder planes (ACT) and their odd partners (DVE; exact:
                # sg is 0/1 so O = sg*E has no extra rounding)
                E, O = [], []
                for t in range(NT):
                    e = pl_pool.tile([P, SW], FP16, name=f"E{t}", tag=f"E{t}")
                    nc.scalar.activation(
                        e[:], m[:], Act.Erf,
                        scale=ERF_SCALE, bias=T32[:, t : t + 1],
                    )
                    E.append(e)
                    o = pl_pool.tile([P, SW], FP16, name=f"O{t}", tag=f"O{t}")
                    nc.vector.tensor_tensor(o[:], sg[:], e[:], Alu.mult)
                    O.append(o)

                halves = []
                for half, g in enumerate(seg):
                    dxx = diag16(ccol16(g, 1), f"dxx{g}", f"dxx{half}")
                    dE = [diag16(ccol16(g, 2 + t), f"dE{g}_{t}",
                                 f"dE{half}_{t}") for t in range(NT)]
                    dO = [diag16(ccol16(g, 2 + NT + t), f"dO{g}_{t}",
                                 f"dO{half}_{t}") for t in range(NT)]
                    Cp = psum_pool.tile([P, W], FP32, name="Cp",
                                        tag=f"Cp{g % 4}")
                    halves.append((g, half * W, dxx, dE, dO, Cp))

                # single PE chain per group:
                # C = axx*x2 + sum be*E + sum bo*O, interleaved across groups
                # and halves per plane
                for (g, goff, dxx, dE, dO, Cp) in halves:
                    for lo in (0, HALF):
                        nc.tensor.matmul(Cp[:, lo : lo + HALF], dxx[:],
                                         x2[:, goff + lo : goff + lo + HALF],
                                         start=True, stop=False)
                for t in range(NT):
                    for (g, goff, dxx, dE, dO, Cp) in halves:
                        for lo in (0, HALF):
                            sl = slice(lo, lo + HALF)
                            xsl = slice(goff + lo, goff + lo + HALF)
                            nc.tensor.matmul(Cp[:, sl], dE[t][:], E[t][:, xsl],
                                             start=False, stop=False)
                            nc.tensor.matmul(Cp[:, sl], dO[t][:], O[t][:, xsl],
                                             start=False, stop=(t == NT - 1))

                for (g, goff, dxx, dE, dO, Cp) in halves:
                    if pending_tail is not None:
                        emit_tail(*pending_tail)
                    pending_tail = (g, goff, xc, Cp)
            emit_tail(*pending_tail)
    nc.compile()
    return nc


def _exact_spline_d(coeff: np.ndarray) -> np.ndarray:
    """Truncated-power coefficients d[j,f]: y(u) = sum_j d_j relu(u-j)^3."""
    d = np.zeros((NB13, F), dtype=np.float64)
    c64 = coeff.astype(np.float64)
    for j in range(NB13):
        for r in range(5):
            n = j - r
            if 0 <= n < coeff.shape[1]:
                d[j] += (-1) ** r * math.comb(4, r) / 6.0 * c64[:, n]
    return d


def _host_fit(coeff: np.ndarray):
    """Weighted, endpoint-constrained LS fit of the folded-erf basis.

    Returns (tabs16 [P, G*NBASIS] fp16, tabs32 [P, NT+G] fp32).
    """
    d = _exact_spline_d(coeff)
    M = 2601
    ug = np.linspace(0.0, 13.0, M)
    # weight: Gaussian density of u = 6.5 x + 6.5 plus clamp point masses
    z = (ug - 6.5) / 6.5
    w = np.exp(-0.5 * z * z)
    tail = math.erfc(1.0 / math.sqrt(2.0)) / 2.0  # P(x > 1)
    w /= w.sum() / (1.0 - 2.0 * tail)
    w[0] += tail
    w[-1] += tail

    yex = np.zeros((M, F))
    for j in range(NB13):
        yex += np.maximum(ug - j, 0.0)[:, None] ** 3 * d[j][None, :]

    # basis columns, mirroring the device fp16 pipeline
    xg32 = z.astype(np.float32)
    xc = np.clip(xg32, -1.0, 1.0).astype(np.float16)
    mm = np.minimum(np.abs(xg32), 1.0).astype(np.float16)
    sg01 = (xc >= 0).astype(np.float64)
    xcf = xc.astype(np.float32)
    from math import erf as _erf
    cols = [np.ones(M), xcf.astype(np.float64),
            (xcf * xcf).astype(np.float16).astype(np.float64)]
    Ecols = []
    for t in range(NT):
        bias = np.float32(-t / ERF_S)
        arg = np.float32(ERF_SCALE) * mm.astype(np.float32) + bias
        e = np.array([_erf(float(v)) for v in arg], dtype=np.float32)
        e16 = e.astype(np.float16).astype(np.float64)
        Ecols.append(e16)
    cols += Ecols
    cols += [sg01 * e for e in Ecols]
    A = np.stack(cols, axis=1)          # (M, 3 + 2*NT)
    B = A.shape[1]

    sw = np.sqrt(w)
    Aw = A * sw[:, None]
    C2 = A[[0, -1], :]
    yc = yex[[0, -1], :]
    AtA = Aw.T @ Aw
    Atb = Aw.T @ (yex * sw[:, None])
    K = np.block([[AtA, C2.T], [C2, np.zeros((2, 2))]])
    sol = np.linalg.lstsq(K, np.vstack([Atb, yc]), rcond=None)[0][:B]
    # sol rows: [a0, ax, axx, be_0..be_6, bo_0..bo_6], per feature

    t16 = np.zeros((P, G * NBASIS), dtype=np.float16)
    t32 = np.zeros((P, NT + G + 1), dtype=np.float32)
    for t in range(NT):
        t32[:, t] = np.float32(-t / ERF_S)
    t32[:, NT + G] = 1.0
    for g in range(G):
        fsl = slice(g * P, (g + 1) * P)
        t16[:, g * NBASIS + 0] = sol[1, fsl].astype(np.float16)
        t16[:, g * NBASIS + 1] = sol[2, fsl].astype(np.float16)
        for t in range(NT):
            t16[:, g * NBASIS + 2 + t] = sol[3 + t, fsl].astype(np.float16)
            t16[:, g * NBASIS + 2 + NT + t] = sol[3 + NT + t, fsl].astype(
                np.float16)
        t32[:, NT + g] = sol[0, fsl].astype(np.float32)
    return t16, t32


def kernel(x: np.ndarray, coeff: np.ndarray) -> np.ndarray:
    x = np.ascontiguousarray(x, dtype=np.float32)
    coeff = np.ascontiguousarray(coeff, dtype=np.float32)
    assert x.shape == (B_FULL, F) and coeff.shape == (F, 10)

    if "nc" not in _CACHE:
        _CACHE["nc"] = _build_nc()
    nc = _CACHE["nc"]

    tabs16, tabs32 = _host_fit(coeff)

    in_maps = []
    for c in range(N_CORES):
        shard = np.ascontiguousarray(x[c * B_CORE : (c + 1) * B_CORE, :].T)
        in_maps.append({"xT": shard, "tabs16": tabs16, "tabs32": tabs32})

    trace = os.environ.get("BSPLINE_TRACE", "0") == "1"
    res = run_bass_kernel_spmd(
        nc, in_maps, core_ids=list(range(N_CORES)), trace=trace
    )
    _CACHE["last_result"] = res

    y = np.empty((B_FULL, F), dtype=np.float32)
    for c in range(N_CORES):
        y[c * B_CORE : (c + 1) * B_CORE, :] = (
            res.results[c]["yT"].astype(np.float32).T
        )
    return y


# revision 27
# speedup vs baseline: 1.5023x; 1.1465x over previous
"""Trainium2 Bass kernel for nn_BSplineActivation.

Math: y[b,f] = sum_n B_n(x[b,f]) * coeff[f,n], cubic B-splines on the uniform
grid linspace(-1,1,14); x clamped to [-1,1]. Per feature, y is a 13-piece C2
cubic in u = 6.5*x + 6.5 with knots at the integers.

Approximation (rel-L2 ~4e-3, gate is 2e-2): per feature f,
  y ~= a0[f] + ax[f]*xc + axx[f]*xc^2
       + sum_{t=0}^{6} be[t,f]*E_t + sum_{t=0}^{6} bo[t,f]*(sgn01 . E_t)
where xc = clip(x,-1,1), m = min(|x|,1), E_t = erf((6.5*m - t)/0.8) and
sgn01 = 1[xc >= 0]. The 13 half-integer-centered erf ladder rungs that fit a
random spline to ~3e-3 fold in symmetric pairs about u=6.5: 7 even planes
E_t(m) (1 ACT op each) span the even part, and the odd part reuses the SAME
planes through a second PSUM chain multiplied by the sign plane in the tail.
Coefficients are per-feature weighted least squares (Gaussian x-density plus
the clamp point masses at x=+-1, exact-interpolation constraints at the two
endpoints), solved on host per call.

Numerics: every basis plane is a smooth function of x evaluated from fp16
tiles, and all fitted coefficients are O(0.3), so fp16 planes/coeffs perturb
y by ~5e-4 (no cancellation anywhere; the ill-conditioned truncated-power
form never materializes on device). PE matmuls run fp16 (1 cyc/row).

Device layout: features on partitions (8 groups of 128 per core), batch on
the free dim; pure data parallel over batch across 8 cores (hosts pass
feature-major transposed shards). Per group-tile [128,1024]:
  DVE: xc16/m16/sgn01 tensor_scalar planes, xg^2, two stt tails
  ACT: 7 erf planes (fused affine, fp16 out)
  Pool: 16 fp16 diag builds (affine_select)
  PE: 16 diag-matmul chains into two PSUM banks-pairs (C and G)
  out: Y fp16, host upcasts.
"""

import math
import os

import numpy as np

import concourse.bacc as bacc
import concourse.bass as bass
import concourse.mybir as mybir
import concourse.tile as tile
from concourse.bass_utils import run_bass_kernel_spmd

N_CORES = 8
B_FULL, F = 8192, 1024
B_CORE = B_FULL // N_CORES  # 1024
P = 128
G = F // P  # 8 feature groups per core
W = B_CORE  # tile width (batch columns)
HALF = 512  # matmul moving-dim limit

NB13 = 13
NT = 7          # erf ladder rungs after symmetry folding (t = 0..6)
ERF_S = 0.8     # erf smoothing width in u units
ERF_SCALE = float(np.float32(6.5 / ERF_S))   # ACT scale on the m plane
NBASIS = 2 + NT + NT  # xc, xc^2, E_t (C chain), E_t (G chain); const via tail

FP32 = mybir.dt.float32
FP16 = mybir.dt.float16
Alu = mybir.AluOpType
Act = mybir.ActivationFunctionType

_CACHE: dict = {}


def _build_nc() -> bass.Bass:
    nc = bacc.Bacc("TRN2", target_bir_lowering=False, debug=False)

    xT = nc.dram_tensor("xT", [F, B_CORE], FP32, kind="ExternalInput")
    # fp16 coefficient table, per feature-group packed columns:
    #   [g*NBASIS + 0]        ax      (xc chain, C psum)
    #   [g*NBASIS + 1]        axx     (xc^2 chain, C psum)
    #   [g*NBASIS + 2 + t]    be[t]   (E_t chain, C psum)
    #   [g*NBASIS + 2+NT + t] bo[t]   (E_t chain, G psum)
    tabs16 = nc.dram_tensor("tabs16", [P, G * NBASIS], FP16, kind="ExternalInput")
    # fp32 table: erf bias columns [0..NT), a0 per group [NT + g], ones [NT+G]
    tabs32 = nc.dram_tensor("tabs32", [P, NT + G + 1], FP32, kind="ExternalInput")
    yT = nc.dram_tensor("yT", [F, B_CORE], FP16, kind="ExternalOutput")

    with tile.TileContext(nc) as tc:
        with (
            tc.tile_pool(name="const", bufs=1) as const_pool,
            tc.tile_pool(name="xdata", bufs=1) as x_pool,
            tc.tile_pool(name="plane", bufs=3) as pl_pool,
            tc.tile_pool(name="diag", bufs=2) as diag_pool,
            tc.tile_pool(name="yout", bufs=2) as y_pool,
            tc.tile_pool(name="psum", bufs=2, space="PSUM") as psum_pool,
        ):
            Xs = []
            for g in range(G):
                X = x_pool.tile([P, W], FP32, name=f"X{g}", tag=f"X{g % 3}")
                nc.sync.dma_start(X[:], xT[g * P : (g + 1) * P, :])
                Xs.append(X)
                if g == 0:
                    T16 = const_pool.tile([P, G * NBASIS], FP16, name="T16")
                    T32 = const_pool.tile([P, NT + G + 1], FP32, name="T32")
                    nc.sync.dma_start(T16[:], tabs16[:])
                    nc.sync.dma_start(T32[:], tabs32[:])

            def ccol16(g, k):
                c = g * NBASIS + k
                return T16[:, c : c + 1]

            def emit_tail(g, xc, sg, Cp, Gp):
                # per 512-half: P1 = ax*xc + C, Tm = sgn01*G, Y = Tm + a0 + P1.
                # Deferred one group so these PSUM-waiting stt ops never stall
                # the ACT erf stream through the in-order DVE queue.
                Y = y_pool.tile([P, W], FP16, name="Y", tag="Y")
                for lo in (0, HALF):
                    sl = slice(lo, lo + HALF)
                    P1 = y_pool.tile([P, HALF], FP32, name="P1", tag="P1")
                    nc.vector.scalar_tensor_tensor(
                        P1[:], xc[:, sl], T16[:, g * NBASIS : g * NBASIS + 1],
                        Cp[:, sl], Alu.mult, Alu.add,
                    )
                    Tm = y_pool.tile([P, HALF], FP32, name="Tm", tag="Tm")
                    nc.vector.scalar_tensor_tensor(
                        Tm[:, :], sg[:, sl], 1.0, Gp[:, sl], Alu.mult, Alu.mult
                    )
                    nc.vector.scalar_tensor_tensor(
                        Y[:, sl], Tm[:, :], T32[:, NT + g : NT + g + 1],
                        P1[:, :], Alu.add, Alu.add,
                    )
                    nc.sync.dma_start(yT[g * P : (g + 1) * P, lo : lo + HALF],
                                      Y[:, sl])

            def diag16(col, name, tag):
                d = diag_pool.tile([P, P], FP16, name=name, tag=tag)
                nc.gpsimd.affine_select(
                    d[:], col.broadcast_to([P, P]),
                    pattern=[[-1, P]], compare_op=Alu.is_equal,
                    fill=0.0, base=0, channel_multiplier=1,
                )
                return d

            pending_tail = None
            for g in range(G):
                X = Xs[g]

                # fp16 planes (DVE tensor_scalar / tensor_tensor)
                xc = pl_pool.tile([P, W], FP16, name="xc", tag="xc")
                nc.vector.tensor_scalar(xc[:], X[:], -1.0, 1.0, Alu.max,
                                        Alu.min)
                m = pl_pool.tile([P, W], FP16, name="m", tag="m")
                nc.vector.tensor_scalar(m[:], X[:], -1.0, 1.0, Alu.mult,
                                        Alu.min)
                nc.vector.tensor_tensor(m[:], xc[:], m[:], Alu.max)
                x2 = pl_pool.tile([P, W], FP16, name="x2", tag="x2")
                nc.vector.tensor_tensor(x2[:], xc[:], xc[:], Alu.mult)
                sg = pl_pool.tile([P, W], FP16, name="sg", tag="sg")
                nc.vector.tensor_scalar(sg[:], xc[:], 0.0, None, Alu.is_ge)

                # 7 erf ladder planes (ACT, fused affine, fp16 out)
                E = []
                for t in range(NT):
                    e = pl_pool.tile([P, W], FP16, name=f"E{t}", tag=f"E{t}")
                    nc.scalar.activation(
                        e[:], m[:], Act.Erf,
                        scale=ERF_SCALE, bias=T32[:, t : t + 1],
                    )
                    E.append(e)

                dxx = diag16(ccol16(g, 1), f"dxx{g}", "dxx")
                dE = [diag16(ccol16(g, 2 + t), f"dE{g}_{t}", f"dE{t}")
                      for t in range(NT)]
                dO = [diag16(ccol16(g, 2 + NT + t), f"dO{g}_{t}", f"dO{t}")
                      for t in range(NT)]

                # PE chains: C = axx*x2 + sum be*E ; G = sum bo*E
                # t-outer, chunk-inner: no ready pass queues behind a pass
                # that waits on a later erf plane
                Cp = psum_pool.tile([P, W], FP32, name="Cp", tag="Cp")
                Gp = psum_pool.tile([P, W], FP32, name="Gp", tag="Gp")
                for lo in (0, HALF):
                    nc.tensor.matmul(Cp[:, lo : lo + HALF], dxx[:],
                                     x2[:, lo : lo + HALF],
                                     start=True, stop=False)
                for t in range(NT):
                    for lo in (0, HALF):
                        sl = slice(lo, lo + HALF)
                        nc.tensor.matmul(Cp[:, sl], dE[t][:], E[t][:, sl],
                                         start=False, stop=(t == NT - 1))
                        nc.tensor.matmul(Gp[:, sl], dO[t][:], E[t][:, sl],
                                         start=(t == 0), stop=(t == NT - 1))

                if pending_tail is not None:
                    emit_tail(*pending_tail)
                pending_tail = (g, xc, sg, Cp, Gp)
            emit_tail(*pending_tail)
    nc.compile()
    return nc


def _exact_spline_d(coeff: np.ndarray) -> np.ndarray:
    """Truncated-power coefficients d[j,f]: y(u) = sum_j d_j relu(u-j)^3."""
    d = np.zeros((NB13, F), dtype=np.float64)
    c64 = coeff.astype(np.float64)
    for j in range(NB13):
        for r in range(5):
            n = j - r
            if 0 <= n < coeff.shape[1]:
                d[j] += (-1) ** r * math.comb(4, r) / 6.0 * c64[:, n]
    return d


def _host_fit(coeff: np.ndarray):
    """Weighted, endpoint-constrained LS fit of the folded-erf basis.

    Returns (tabs16 [P, G*NBASIS] fp16, tabs32 [P, NT+G] fp32).
    """
    d = _exact_spline_d(coeff)
    M = 2601
    ug = np.linspace(0.0, 13.0, M)
    # weight: Gaussian density of u = 6.5 x + 6.5 plus clamp point masses
    z = (ug - 6.5) / 6.5
    w = np.exp(-0.5 * z * z)
    tail = math.erfc(1.0 / math.sqrt(2.0)) / 2.0  # P(x > 1)
    w /= w.sum() / (1.0 - 2.0 * tail)
    w[0] += tail
    w[-1] += tail

    yex = np.zeros((M, F))
    for j in range(NB13):
        yex += np.maximum(ug - j, 0.0)[:, None] ** 3 * d[j][None, :]

    # basis columns, mirroring the device fp16 pipeline
    xg32 = z.astype(np.float32)
    xc = np.clip(xg32, -1.0, 1.0).astype(np.float16)
    mm = np.minimum(np.abs(xg32), 1.0).astype(np.float16)
    sg01 = (xc >= 0).astype(np.float64)
    xcf = xc.astype(np.float32)
    from math import erf as _erf
    cols = [np.ones(M), xcf.astype(np.float64),
            (xcf * xcf).astype(np.float16).astype(np.float64)]
    Ecols = []
    for t in range(NT):
        bias = np.float32(-t / ERF_S)
        arg = np.float32(ERF_SCALE) * mm.astype(np.float32) + bias
        e = np.array([_erf(float(v)) for v in arg], dtype=np.float32)
        e16 = e.astype(np.float16).astype(np.float64)
        Ecols.append(e16)
    cols += Ecols
    cols += [sg01 * e for e in Ecols]
    A = np.stack(cols, axis=1)          # (M, 3 + 2*NT)
    B = A.shape[1]

    sw = np.sqrt(w)
    Aw = A * sw[:, None]
    C2 = A[[0, -1], :]
    yc = yex[[0, -1], :]
    AtA = Aw.T @ Aw
    Atb = Aw.T @ (yex * sw[:, None])
    K = np.block([[AtA, C2.T], [C2, np.zeros((2, 2))]])
    sol = np.linalg.lstsq(K, np.vstack([Atb, yc]), rcond=None)[0][:B]
    # sol rows: [a0, ax, axx, be_0..be_6, bo_0..bo_6], per feature

    t16 = np.zeros((P, G * NBASIS), dtype=np.float16)
    t32 = np.zeros((P, NT + G + 1), dtype=np.float32)
    for t in range(NT):
        t32[:, t] = np.float32(-t / ERF_S)
    t32[:, NT + G] = 1.0
    for g in range(G):
        fsl = slice(g * P, (g + 1) * P)
        t16[:, g * NBASIS + 0] = sol[1, fsl].astype(np.float16)
        t16[:, g * NBASIS + 1] = sol[2, fsl].astype(np.float16)
        for t in range(NT):
            t16[:, g * NBASIS + 2 + t] = sol[3 + t, fsl].astype(np.float16)
            t16[:, g * NBASIS + 2 + NT + t] = sol[3 + NT + t, fsl].astype(
                np.float16)
        t32[:, NT + g] = sol[0, fsl].astype(np.float32)
    return t16, t32


def kernel(x: np.ndarray, coeff: np.ndarray) -> np.ndarray:
    x = np.ascontiguousarray(x, dtype=np.float32)
    coeff = np.ascontiguousarray(coeff, dtype=np.float32)
    assert x.shape == (B_FULL, F) and coeff.shape == (F, 10)

    if "nc" not in _CACHE:
        _CACHE["nc"] = _build_nc()
    nc = _CACHE["nc"]

    tabs16, tabs32 = _host_fit(coeff)

    in_maps = []
    for c in range(N_CORES):
        shard = np.ascontiguousarray(x[c * B_CORE : (c + 1) * B_CORE, :].T)
        in_maps.append({"xT": shard, "tabs16": tabs16, "tabs32": tabs32})

    trace = os.environ.get("BSPLINE_TRACE", "0") == "1"
    res = run_bass_kernel_spmd(
        nc, in_maps, core_ids=list(range(N_CORES)), trace=trace
    )
    _CACHE["last_result"] = res

    y = np.empty((B_FULL, F), dtype=np.float32)
    for c in range(N_CORES):
        y[c * B_CORE : (c + 1) * B_CORE, :] = (
            res.results[c]["yT"].astype(np.float32).T
        )
    return y
